# revision 7
# baseline (speedup 1.0000x reference)
"""BPR loss kernel for Trainium2 (8 NeuronCores, SPMD data-parallel).

Problem:
    predict: (4096, 100000) f32, pos_idx/neg_idx: (4096, 50) int
    loss = sum_b -mean_k logsigmoid(predict[b, pos_idx[b,k]] - predict[b, neg_idx[b,k]])

Strategy:
    Shard batch rows across 8 cores (512 rows each). On each core, gather the
    2*512*50 = 51200 needed scalars from the predict shard with an indirect
    DMA (flat int32 element indices precomputed on host), then compute
    -logsigmoid via Sigmoid+Ln on the scalar engine with a fused per-partition
    row-sum (accum_out). Each core returns 128 partial sums; host reduces.
    This reads ~0.2% of predict instead of the full 1.6 GB.

Raw bass (no Tile): the Tile tail drain accumulates >4 sem waits on one
instruction, which the walrus codegen rejects ("Too many sync wait commands").
"""

import numpy as np

import concourse.bass as bass
from concourse import mybir
from concourse.bass_utils import run_bass_kernel_spmd

B, N, K = 4096, 100000, 50
NCORES = 8
RB = B // NCORES          # 512 rows per core
P = 128                   # SBUF partitions
RPP = RB // P             # 4 rows per partition
FREE = RPP * K            # 200 gathered scalars per partition (pos or neg)

_NC_CACHE = None


def build_bass():
    nc = bass.Bass()
    predict = nc.declare_dram_parameter(
        "predict", [RB * N, 1], mybir.dt.float32, isOutput=False
    )
    idx = nc.declare_dram_parameter("idx", [P, 2 * FREE], mybir.dt.int32, isOutput=False)
    out = nc.declare_dram_parameter("out", [P, 1], mybir.dt.float32, isOutput=True)

    f32 = mybir.dt.float32
    with (
        nc.sbuf_tensor([P, 2 * FREE], mybir.dt.int32) as idx_t,
        nc.sbuf_tensor([P, 2 * FREE], f32) as vals,
        nc.sbuf_tensor([P, FREE], f32) as d,
        nc.sbuf_tensor([P, FREE], f32) as sig,
        nc.sbuf_tensor([P, FREE], f32) as ls,
        nc.sbuf_tensor([P, 1], f32) as part,
        nc.semaphore("dma_sem") as dma_sem,
        nc.semaphore("c_sem") as c_sem,
        nc.Block() as block,
    ):

        @block.gpsimd
        def _(gpsimd):
            gpsimd.dma_start(out=idx_t[:], in_=idx[:]).then_inc(dma_sem, 16)
            gpsimd.wait_ge(dma_sem, 16)
            gpsimd.indirect_dma_start(
                out=vals[:],
                out_offset=None,
                in_=predict[:],
                in_offset=bass.IndirectOffsetOnAxis(ap=idx_t[:], axis=0),
            ).then_inc(dma_sem, 16)
            gpsimd.wait_ge(c_sem, 3)
            gpsimd.dma_start(out=out[:], in_=part[:]).then_inc(dma_sem, 16)
            gpsimd.wait_ge(dma_sem, 48)

        @block.vector
        def _(vector):
            vector.wait_ge(dma_sem, 32)
            nc.vector.tensor_tensor(
                out=d[:],
                in0=vals[:, :FREE],
                in1=vals[:, FREE:],
                op=mybir.AluOpType.subtract,
            ).then_inc(c_sem, 1)

        @block.scalar
        def _(scalar):
            scalar.wait_ge(c_sem, 1)
            nc.scalar.activation(
                out=sig[:], in_=d[:], func=mybir.ActivationFunctionType.Sigmoid
            ).then_inc(c_sem, 1)
            scalar.wait_ge(c_sem, 2)
            nc.scalar.activation(
                out=ls[:],
                in_=sig[:],
                func=mybir.ActivationFunctionType.Ln,
                accum_out=part[:],
            ).then_inc(c_sem, 1)

    return nc


def make_in_maps(predict, pos_idx, neg_idx):
    predict = np.ascontiguousarray(np.asarray(predict), dtype=np.float32)
    pos_idx = np.asarray(pos_idx)
    neg_idx = np.asarray(neg_idx)

    in_maps = []
    row_off = (np.arange(RB, dtype=np.int64)[:, None] * N)  # (512, 1)
    for c in range(NCORES):
        r0 = c * RB
        fp = (row_off + pos_idx[r0 : r0 + RB].astype(np.int64)).astype(np.int32)
        fn = (row_off + neg_idx[r0 : r0 + RB].astype(np.int64)).astype(np.int32)
        idx_all = np.concatenate(
            [fp.reshape(P, FREE), fn.reshape(P, FREE)], axis=1
        )  # (128, 400)
        in_maps.append(
            {
                "predict": predict[r0 : r0 + RB].reshape(-1, 1),
                "idx": np.ascontiguousarray(idx_all),
            }
        )
    return in_maps


def run(predict, pos_idx, neg_idx, trace=False, **kwargs):
    global _NC_CACHE
    if _NC_CACHE is None:
        _NC_CACHE = build_bass()
    nc = _NC_CACHE
    in_maps = make_in_maps(predict, pos_idx, neg_idx)
    res = run_bass_kernel_spmd(nc, in_maps, list(range(NCORES)), trace=trace, **kwargs)
    total = np.float64(0.0)
    for r in res.results:
        total += r["out"].astype(np.float64).sum()
    out = np.float32(-total / K)
    return out, res


def kernel(predict, pos_idx, neg_idx):
    out, _ = run(predict, pos_idx, neg_idx, trace=False)
    return out


# revision 9
# speedup vs baseline: 1.5036x; 1.5036x over previous
"""BPR loss kernel for Trainium2 (8 NeuronCores, SPMD data-parallel).

Problem:
    predict: (4096, 100000) f32, pos_idx/neg_idx: (4096, 50) int
    loss = sum_b -mean_k logsigmoid(predict[b, pos_idx[b,k]] - predict[b, neg_idx[b,k]])

Strategy (per core, 512 rows):
    - host precomputes flat int32 element indices into the core's predict shard
    - SP/HWDGE loads the 128x400 index tile into SBUF
    - one SWDGE indirect DMA gathers all 51200 scalars (pos block | neg block)
    - -logsigmoid(d) = ln(1 + exp(-d)): DVE subtract, ACT Exp, DVE +1, ACT Ln
      with fused per-partition row-sum (accum_out). Exp and Ln share one ACT
      table set (natural_log_exp_and_others), pre-warmed by a dummy op during
      the DMA phase, so zero table loads sit on the critical path.
    - PE dots the 128 partial sums with a ones vector -> single f32 in PSUM
    - DVE copies PSUM->SBUF, SP stores 4 bytes to DRAM (single descriptor; a
      128-partition store paid ~7us in per-engine HBM completion receipts)
    Host sums the 8 core scalars and divides by K.

Raw bass (no Tile): the Tile tail drain accumulates >4 sem waits on one
instruction, which the walrus codegen rejects ("Too many sync wait commands").
"""

import numpy as np

import concourse.bass as bass
from concourse import mybir
from concourse.bass_utils import run_bass_kernel_spmd

B, N, K = 4096, 100000, 50
NCORES = 8
RB = B // NCORES          # 512 rows per core
P = 128                   # SBUF partitions
RPP = RB // P             # 4 rows per partition
FREE = RPP * K            # 200 gathered scalars per partition (pos or neg)

_NC_CACHE = None


def build_bass():
    nc = bass.Bass()
    predict = nc.declare_dram_parameter(
        "predict", [RB * N, 1], mybir.dt.float32, isOutput=False
    )
    idx = nc.declare_dram_parameter("idx", [P, 2 * FREE], mybir.dt.int32, isOutput=False)
    out = nc.declare_dram_parameter("out", [1, 1], mybir.dt.float32, isOutput=True)

    f32 = mybir.dt.float32
    AF = mybir.ActivationFunctionType
    ones = nc.const_aps.aps[(f32, 1.0)]   # [128, 1], memset in preamble
    zero = nc.const_aps.aps[(f32, 0.0)]   # [128, 1]

    with (
        nc.sbuf_tensor([P, 2 * FREE], mybir.dt.int32) as idx_t,
        nc.sbuf_tensor([P, 2 * FREE], f32) as vals,
        nc.sbuf_tensor([P, FREE], f32) as d,
        nc.sbuf_tensor([P, FREE], f32) as e,
        nc.sbuf_tensor([P, FREE], f32) as u,
        nc.sbuf_tensor([P, FREE], f32) as act_out,
        nc.sbuf_tensor([P, 1], f32) as part,
        nc.sbuf_tensor([P, 1], f32) as dummy,
        nc.sbuf_tensor([1, 1], f32) as scalar_out,
        nc.psum_tensor([1, 1], f32) as psum_s,
        nc.semaphore("s_idx") as s_idx,
        nc.semaphore("s_g") as s_g,
        nc.semaphore("c_sem") as c_sem,
        nc.semaphore("s_out") as s_out,
        nc.Block() as block,
    ):

        @block.sync
        def _(sync):
            sync.dma_start(out=idx_t[:], in_=idx[:]).then_inc(s_idx, 16)
            sync.wait_ge(c_sem, 6)
            sync.dma_start(out=out[:], in_=scalar_out[:]).then_inc(s_out, 16)
            sync.wait_ge(s_out, 16)

        @block.gpsimd
        def _(gpsimd):
            gpsimd.wait_ge(s_idx, 16)
            gpsimd.indirect_dma_start(
                out=vals[:],
                out_offset=None,
                in_=predict[:],
                in_offset=bass.IndirectOffsetOnAxis(ap=idx_t[:], axis=0),
            ).then_inc(s_g, 16)

        @block.vector
        def _(vector):
            vector.wait_ge(s_g, 16)
            # d = neg - pos; then ln(1+exp(d)) = -logsigmoid(pos-neg)
            nc.vector.tensor_tensor(
                out=d[:],
                in0=vals[:, FREE:],
                in1=vals[:, :FREE],
                op=mybir.AluOpType.subtract,
            ).then_inc(c_sem, 1)
            vector.wait_ge(c_sem, 2)
            nc.vector.tensor_scalar_add(u[:], e[:], 1.0).then_inc(c_sem, 1)
            vector.wait_ge(c_sem, 5)
            nc.vector.tensor_copy(out=scalar_out[:], in_=psum_s[:]).then_inc(c_sem, 1)

        @block.scalar
        def _(scalar):
            # dummy op pulls the exp/ln table set in while the DMAs run
            nc.scalar.activation(out=dummy[:], in_=zero, func=AF.Exp)
            scalar.wait_ge(c_sem, 1)
            nc.scalar.activation(out=e[:], in_=d[:], func=AF.Exp).then_inc(c_sem, 1)
            scalar.wait_ge(c_sem, 3)
            nc.scalar.activation(
                out=act_out[:], in_=u[:], func=AF.Ln, accum_out=part[:]
            ).then_inc(c_sem, 1)

        @block.tensor
        def _(tensor):
            tensor.wait_ge(c_sem, 4)
            nc.tensor.matmul(
                out=psum_s[:], lhsT=part[:], rhs=ones, start=True, stop=True
            ).then_inc(c_sem, 1)

    return nc


def make_in_maps(predict, pos_idx, neg_idx):
    predict = np.ascontiguousarray(np.asarray(predict), dtype=np.float32)
    pos_idx = np.asarray(pos_idx)
    neg_idx = np.asarray(neg_idx)

    in_maps = []
    row_off = (np.arange(RB, dtype=np.int64)[:, None] * N)  # (512, 1)
    for c in range(NCORES):
        r0 = c * RB
        fp = (row_off + pos_idx[r0 : r0 + RB].astype(np.int64)).astype(np.int32)
        fn = (row_off + neg_idx[r0 : r0 + RB].astype(np.int64)).astype(np.int32)
        idx_all = np.concatenate(
            [fp.reshape(P, FREE), fn.reshape(P, FREE)], axis=1
        )  # (128, 400)
        in_maps.append(
            {
                "predict": predict[r0 : r0 + RB].reshape(-1, 1),
                "idx": np.ascontiguousarray(idx_all),
            }
        )
    return in_maps


def run(predict, pos_idx, neg_idx, trace=False, **kwargs):
    global _NC_CACHE
    if _NC_CACHE is None:
        _NC_CACHE = build_bass()
    nc = _NC_CACHE
    in_maps = make_in_maps(predict, pos_idx, neg_idx)
    res = run_bass_kernel_spmd(nc, in_maps, list(range(NCORES)), trace=trace, **kwargs)
    total = np.float64(0.0)
    for r in res.results:
        total += np.float64(r["out"][0, 0])
    out = np.float32(total / K)
    return out, res


def kernel(predict, pos_idx, neg_idx):
    out, _ = run(predict, pos_idx, neg_idx, trace=False)
    return out
